# revision 1
# baseline (speedup 1.0000x reference)
"""Distributed causal attention (softmax over the QUERY axis) on 8 TRN2
NeuronCores, written in Bass/Tile.

Sharding: the reference normalizes softmax over the query axis (axis=1), so
each key-column's softmax is independent. We therefore shard the KEY axis:
core pair (2b, 2b+1) handles batch b, with even cores owning even 128-row
k-tiles and odd cores owning odd k-tiles. The interleaving makes the
causal-sparsity-aware SPMD instruction graph identical on all cores (the one
asymmetric bit - the diagonal mask - is per-core input data). The Wk projection is merged into the q side on the host
(scores = q (Wq^T Wk) k^T), eliminating the k-projection phase entirely;
the merged projection is split across each pair and exchanged through
pair-shared HBM (addr_space="Shared") with a zero-payload pair AllReduce as
the cross-core fence. The q-projection (phase Q) runs fully in fp8 DoubleRow
(q and 32*M both e4m3; rel-err 1.66e-2 vs the 2e-2 gate, predicted exactly
by a host numpy simulation of the quantization chain); the score matmuls
contract raw fp8 k against the fp8 qh as DoubleRow pairs (256-row
contraction); the scale is applied inside the exp activation so all fp8
operands stay in e4m3's normal range. The v-projection and PV stay bf16
(fp8 there adds ~3-4% error, over the gate). Inputs are host-packed
partition-major and loaded as single full-tile DMAs (8-16KB contiguous
descriptors; HWDGE descriptor-gen is ~600ns/instruction and ring bandwidth
scales with descriptor size). Timing is dominated by the CC collective
chain: the CC engine boots ~21us into the kernel, registers the first
collective ~14us later, and starts its first mesh 11.06us after the latest
pre-mesh trigger; the warm collective absorbs this under phases Q/B so
phase D starts ~63-70us. The host sums the two partial PV outputs per
batch pair.
"""

from contextlib import ExitStack

import numpy as np
import ml_dtypes

import concourse.bass as bass
from concourse import bacc
import concourse.tile as tile
import concourse.mybir as mybir
from concourse.bass_utils import run_bass_kernel_spmd
from concourse.tile import ScopedClock

BATCH = 4


def _fast_drain_and_barrier(self, tick_clock, wait_clock):
    """Tile kernel-tail with sem-only all-engine barriers (the default
    drain+butterfly pair costs ~8us); the explicit sync.drain with the global
    clock waits already covers all tracked work."""
    drain_inst = self.nc.sync.drain()
    wait_clock.add_sem_waits(
        drain_inst.ins, ScopedClock({None: tick_clock.global_clock})
    )
    self.nc.all_engine_barrier(sem_only=True)
    assert self.sems is not None
    popped = self.nc._tile_sem_poison_stack.pop()
    assert popped is self._sem_poison
    self.nc.clear_and_free_semaphores(list(self.sems.allocated().values()))
    self.nc.all_engine_barrier(sem_only=True)


tile.TileContext._drain_and_barrier = _fast_drain_and_barrier

P = 128
SEQ = 2048
E = 1024
H = 1024
KL = 1024          # k columns per core (16 tiles / 2 cores * 128)
NE = E // P        # 8
NH = H // P        # 8
NKS = KL // P      # 8 k slots per core
NQT = SEQ // P     # 16 q tiles
NB = 512           # matmul free-dim / psum bank
MASK_NEG = -51200.0  # pre-exp-scale; exp applies 1/1024 -> effective -50
N_WARMUP = 12   # longer warmups regress: sustained dummy PE activity trips
                # the HAM/DVFS throttle before the real phases begin

BF16 = mybir.dt.bfloat16
FP8 = mybir.dt.float8e4
F32 = mybir.dt.float32
nbf16 = ml_dtypes.bfloat16
nfp8 = ml_dtypes.float8_e4m3


def slot_chunks(j):
    """(ext, [(off, width), ...]) q-chunks for score slot j (relative to 256j)."""
    ext = SEQ - 256 * j
    chunks = []
    off = 0
    if j % 2 == 1:
        chunks.append((0, 256))
        off = 256
    while off < ext:
        chunks.append((off, NB))
        off += NB
    return ext, chunks


def build_nc():
    nc = bacc.Bacc("TRN2", target_bir_lowering=False, debug=False, num_devices=8)
    # Inputs arrive pre-packed partition-major ([128, NE*width]: partition p,
    # free e*width+x  <->  logical row e*128+p, col x) so each tensor loads
    # with a couple of DMAs of 128 large contiguous descriptors instead of
    # many 2KB-descriptor transfers that bottleneck the HWDGE rings.
    NEP = NE // 2
    qT = nc.dram_tensor("qT", [P, NEP, 2, SEQ // 2], FP8, kind="ExternalInput").ap()
    kT = nc.dram_tensor("kT", [P, 2, NEP * KL], FP8, kind="ExternalInput").ap()
    vT = nc.dram_tensor("vT", [P, NE * KL], BF16, kind="ExternalInput").ap()
    wqT = nc.dram_tensor("wqT", [P, NEP, 2, H], FP8, kind="ExternalInput").ap()
    wvT = nc.dram_tensor("wvT", [P, NE * H], BF16, kind="ExternalInput").ap()
    mask = nc.dram_tensor("mask", [P, 256], F32, kind="ExternalInput").ap()
    out = nc.dram_tensor("out", [SEQ, H], BF16, kind="ExternalOutput").ap()

    with tile.TileContext(nc) as tc, ExitStack() as ctx:
        wpool = ctx.enter_context(tc.tile_pool(name="w", bufs=1))
        ktv = ctx.enter_context(tc.tile_pool(name="ktv", bufs=1))
        qtp = ctx.enter_context(tc.tile_pool(name="qtp", bufs=1))
        qhpool = ctx.enter_context(tc.tile_pool(name="qh", bufs=1))
        khpool = ctx.enter_context(tc.tile_pool(name="kh", bufs=NH // 2))
        vhpool = ctx.enter_context(tc.tile_pool(name="vh", bufs=NKS))
        prpool = ctx.enter_context(tc.tile_pool(name="pr", bufs=1))
        smpool = ctx.enter_context(tc.tile_pool(name="sm", bufs=1))
        ostpool = ctx.enter_context(tc.tile_pool(name="ost", bufs=10))
        qhh_pool = ctx.enter_context(tc.tile_pool(name="qhh", bufs=1))
        dram = ctx.enter_context(tc.tile_pool(name="dram", bufs=1, space="DRAM"))
        psum = ctx.enter_context(tc.tile_pool(name="ps", bufs=8, space="PSUM"))

        # ---- PE warmup: dummy matmuls with no input deps run during the
        # initial DMA wait, releasing the HAM clock throttle early ----
        # Warm pair-collective with an UNINITIALIZED input (no data deps, so
        # gpsimd triggers it at ~8us). CC firmware boots ~21us in, registers
        # the warm ~14us later, and starts its mesh 11.06us after the LATEST
        # pre-mesh trigger. The real fence's trigger (gated on the qh store,
        # ~44us with one-shot M/q) lands after the warm's mesh has begun, so
        # its mesh runs immediately after with no fresh countdown — phase D
        # starts right as phase B drains.
        fence_in = dram.tile([P, 1], F32, tag="fin", name="fence_in")
        fence_out = dram.tile([P, 1], F32, tag="fout", name="fence_out")
        warm_out = dram.tile([P, 1], F32, tag="wout", name="warm_out")
        nc.gpsimd.collective_compute(
            "AllReduce",
            mybir.AluOpType.add,
            replica_groups=[[0, 1], [2, 3], [4, 5], [6, 7]],
            ins=[fence_in.opt()],
            outs=[warm_out.opt()],
        )
        wrm = smpool.tile([P, 256], BF16, tag="wrm", name="wrm")
        nc.vector.memset(wrm[:], 0.0)
        wps = psum.tile([P, 256], F32, tag="ps", name="wps")
        for i in range(N_WARMUP):
            nc.tensor.matmul(wps[:], lhsT=wrm[:, :P], rhs=wrm[:],
                             start=(i == 0), stop=(i == N_WARMUP - 1))

        # ---- mask ----
        msk = smpool.tile([P, 256], F32, tag="msk", name="msk")
        nc.sync.dma_start(msk[:], mask[:])

        # Alternate input DMAs between the SP and ACT HWDGE rings so
        # descriptor generation parallelizes.
        def in_dma(e, *args):
            return (nc.sync if e % 2 == 0 else nc.scalar).dma_start(*args)

        # ---- input DMAs, phase Q: per-ep chunks (2KB descriptors) landing
        # in the exact ep-consumption order of the ep-OUTER loop below ----
        # so each ep-chunk is one DMA of 2KB-contiguous descriptors, landing in
        # the exact ep-accumulation order of the loop below ----
        NEP = NE // 2
        SEQ2 = SEQ // 2
        wq_sb = wpool.tile([P, NEP, 2, H], FP8, tag="wq", name="wq_sb")
        qt_sb = qtp.tile([P, NEP, 2, SEQ2], FP8, tag="qt", name="qt_sb")
        # 2 chunks of 2 eps each: 4KB-contiguous descriptors move ~2x the
        # bandwidth of 1-ep (2KB) chunks on the throttled uncore while still
        # arriving in ep order for the ep-outer consumption below.
        for c in range(2):
            in_dma(c, wq_sb[:, 2 * c:2 * c + 2, :, :],
                   wqT[:, 2 * c:2 * c + 2, :, :])
            in_dma(c + 1, qt_sb[:, 2 * c:2 * c + 2, :, :],
                   qT[:, 2 * c:2 * c + 2, :, :])

        # ---- phase Q: this core's half of qhT ([h, SEQ/2]), fp8 DoubleRow
        # (256-row contraction per matmul). ep is the OUTERMOST loop across
        # two h-groups of 4 (8 concurrent PSUM banks = 2 qb x 4 h), so the
        # tensor engine consumes the per-ep input chunks strictly in arrival
        # order: phase Q starts as soon as the first ep chunk lands (~12us)
        # and never waits for the full M/q transfer. Output goes into ONE
        # packed tile (free = h*SEQ2 + q) so the pair exchange is a single
        # large-descriptor store.
        qhh_all = qhh_pool.tile([P, NH * SEQ2], FP8, tag="qhh", name="qhh_all")
        NQB = SEQ // 2 // NB  # 2
        for hg in range(2):
            pts = [[psum.tile([P, NB], F32, tag="ps", name=f"pq{hg}_{qb}_{hh}")
                    for hh in range(4)] for qb in range(NQB)]
            for ep in range(NEP):
                for qb in range(NQB):
                    for hh in range(4):
                        h = hg * 4 + hh
                        nc.tensor.matmul(
                            pts[qb][hh][:],
                            lhsT=wq_sb[:, ep:ep + 1, :, h * P:(h + 1) * P]
                                .squeeze(1),
                            rhs=qt_sb[:, ep:ep + 1, :, qb * NB:(qb + 1) * NB]
                                .squeeze(1),
                            start=(ep == 0),
                            stop=(ep == NEP - 1),
                            perf_mode=mybir.MatmulPerfMode.DoubleRow,
                        )
            for qb in range(NQB):
                for hh in range(4):
                    h = hg * 4 + hh
                    nc.vector.tensor_copy(
                        qhh_all[:, h * SEQ2 + qb * NB:h * SEQ2 + (qb + 1) * NB],
                        pts[qb][hh][:])

        # ---- pair exchange of the qh halves via pair-shared HBM ----
        # Cores 2k/2k+1 share an HBM domain; addr_space="Shared" DRAM is one
        # physical buffer per pair. Each core stores its qh half at a
        # parity-dependent row offset, a zero-payload pair AllReduce acts as
        # the cross-core fence, then both halves stream back at full HBM
        # bandwidth (vs ~60us through the ncfw AllGather data path).
        shm_all = dram.tile([P, 2 * NH * (SEQ // 2)], FP8, tag="shm", name="shm_all",
                            addr_space="Shared")
        # One store of the whole packed half at a parity-dependent free offset
        # (8KB contiguous descriptors on the SP HWDGE ring), then two large
        # contiguous reads after the fence. qh_all free layout:
        # (half, ep, ko, qlocal) with e'-tile = 2*ep+ko, q = half*SEQ2+qlocal.
        HB = NH * SEQ2  # 8192 elems: one core-half of qh
        my_off = (nc.gpsimd.partition_id() % 2) * HB
        st = nc.gpsimd.dma_start(shm_all[:, bass.ds(my_off, HB)], qhh_all[:])
        cc = nc.gpsimd.collective_compute(
            "AllReduce",
            mybir.AluOpType.add,
            replica_groups=[[0, 1], [2, 3], [4, 5], [6, 7]],
            ins=[fence_in.opt()],
            outs=[fence_out.opt()],
        )
        tile.add_dep_helper(cc.ins, st.ins, reason="fence waits on shm store")
        # qh_all free layout (hf, ep, ko, qlocal): read back as 4 DMAs of
        # 4KB-contiguous descriptors, split across both HWDGE queues so
        # descriptor generation for the two halves runs in parallel.
        qh_all = qhpool.tile([P, 2, NEP, 2, SEQ2], FP8, tag="qh2", name="qh_all")
        HB2 = HB // 2
        for rq in range(4):
            r = in_dma(rq, qh_all[:, rq // 2:rq // 2 + 1, (rq % 2) * (NEP // 2):
                               (rq % 2 + 1) * (NEP // 2), :, :],
                       shm_all[:, rq * HB2:(rq + 1) * HB2])
            tile.add_dep_helper(r.ins, cc.ins, reason="qh read waits on fence")

        # ---- input DMAs, phase B (vh) + raw k, one full-tile DMA each,
        # queued before kT on the sync ring (kT is only needed at phase D) ----
        wv_sb = wpool.tile([P, NE * H], BF16, tag="wv", name="wv_sb")
        vt_sb = ktv.tile([P, NE * KL], BF16, tag="vt", name="vt_sb")
        in_dma(0, vt_sb[:], vT[:])
        in_dma(1, wv_sb[:], wvT[:])
        kt2 = ktv.tile([P, 2, NEP * KL], FP8, tag="kt2", name="kt2")
        in_dma(0, kt2[:, :, :], kT[:, :, :])

        # ---- phase B: vh[kl, h] (e-outer, 8 concurrent banks) ----
        vh_sb = [vhpool.tile([P, H], BF16, tag="vh", name=f"vh{j}")
                 for j in range(NKS)]
        for hb in range(H // NB):
            pts = [psum.tile([P, NB], F32, tag="ps", name=f"pp_vh{j}_{hb}")
                   for j in range(NKS)]
            for e in range(NE):
                for j in range(NKS):
                    nc.tensor.matmul(
                        pts[j][:],
                        lhsT=vt_sb[:, e * KL + j * P:e * KL + (j + 1) * P],
                        rhs=wv_sb[:, e * H + hb * NB:e * H + (hb + 1) * NB],
                        start=(e == 0),
                        stop=(e == NE - 1),
                    )
            for j in range(NKS):
                nc.vector.tensor_copy(vh_sb[j][:, hb * NB:(hb + 1) * NB], pts[j][:])

        # ---- phase D: scoresT -> exp -> den -> scale vh ----
        pr_sb = []
        for j in range(NKS):
            ext, chunks = slot_chunks(j)
            q0 = 256 * j
            pr = prpool.tile([P, ext], BF16, tag=f"pr{j}", name=f"pr{j}")
            accs = smpool.tile([P, len(chunks)], F32, tag=f"acc{j}", name=f"acc{j}")
            for ci, (off, w) in enumerate(chunks):
                pt = psum.tile([P, NB], F32, tag="ps", name=f"sp{j}_{ci}")
                hf, qlo = (q0 + off) // SEQ2, (q0 + off) % SEQ2
                for ep in range(NEP):
                    nc.tensor.matmul(
                        pt[:, :w],
                        lhsT=kt2[:, :, ep * KL + j * P:ep * KL + (j + 1) * P],
                        rhs=qh_all[:, hf:hf + 1, ep:ep + 1, :, qlo:qlo + w]
                            .squeeze(1).squeeze(1),
                        start=(ep == 0),
                        stop=(ep == NEP - 1),
                        perf_mode=mybir.MatmulPerfMode.DoubleRow,
                    )
                if off == 0:
                    nc.vector.tensor_add(pt[:, :256], pt[:, :256], msk[:])
                nc.scalar.activation(
                    pr[:, off:off + w],
                    pt[:, :w],
                    mybir.ActivationFunctionType.Exp,
                    scale=float(1.0 / (np.sqrt(H) * 32.0)),
                    accum_out=accs[:, ci:ci + 1],
                )
            den = smpool.tile([P, 1], F32, tag=f"den{j}", name=f"den{j}")
            nc.vector.tensor_reduce(
                den[:], accs[:], axis=mybir.AxisListType.X, op=mybir.AluOpType.add
            )
            rec = smpool.tile([P, 1], F32, tag=f"rec{j}", name=f"rec{j}")
            nc.vector.reciprocal(rec[:], den[:])
            nc.vector.tensor_scalar_mul(vh_sb[j][:], vh_sb[j][:], rec[:])
            pr_sb.append(pr)

        # ---- phase E: PV + output ----
        # Interleave big-t (many accumulation MMs) and small-t (few) groups so
        # the DVE copy + out-DMA drain keeps pace with PSUM-bank production.
        pv_order = []
        lo, hi = 0, NQT - 1
        while lo <= hi:
            pv_order.append(hi)
            if lo < hi:
                pv_order.append(lo)
            hi -= 1
            lo += 1
        for t in pv_order:
            jmax = t // 2
            for hb in range(H // NB):
                pt = psum.tile([P, NB], F32, tag="ps", name=f"pv{t}_{hb}")
                for j in range(jmax + 1):
                    off = t * P - 256 * j
                    nc.tensor.matmul(
                        pt[:],
                        lhsT=pr_sb[j][:, off:off + P],
                        rhs=vh_sb[j][:, hb * NB:(hb + 1) * NB],
                        start=(j == 0),
                        stop=(j == jmax),
                    )
                ot = ostpool.tile([P, NB], BF16, tag="ost", name=f"ot{t}_{hb}")
                if t == pv_order[-1]:
                    # final tile: halve copy+DMA so the drain tail pipelines
                    for ha in range(2):
                        sl = slice(ha * (NB // 2), (ha + 1) * (NB // 2))
                        nc.vector.tensor_copy(ot[:, sl], pt[:, sl])
                        eng2 = nc.sync if (hb + ha) % 2 == 0 else nc.scalar
                        eng2.dma_start(
                            out[t * P:(t + 1) * P,
                                hb * NB + ha * (NB // 2):
                                hb * NB + (ha + 1) * (NB // 2)],
                            ot[:, sl],
                        )
                    continue
                nc.vector.tensor_copy(ot[:], pt[:])
                n_done = len([x for x in pv_order[:pv_order.index(t) + 1]])
                eng = nc.gpsimd if n_done <= NQT - 4 else (
                    nc.sync if hb == 0 else nc.scalar)
                eng.dma_start(
                    out[t * P:(t + 1) * P, hb * NB:(hb + 1) * NB], ot[:]
                )

    nc.compile()
    return nc


# ---------------- host-side prep ----------------

def core_k_tiles(parity):
    return list(range(parity, 16, 2))


def _pack(m):
    """[NE*128, X] -> [128, NE*X]: partition-major so DMA descriptors are
    large and contiguous (row e*128+p, col x) -> (p, e*X+x)."""
    r, x = m.shape
    return np.ascontiguousarray(
        m.reshape(NE, P, x).transpose(1, 0, 2).reshape(P, NE * x)
    )


def _pack_pair(m):
    """[NE*128, X] -> [128, 2, (NE/2)*X] fp8 pair-interleave for DoubleRow:
    (p, eo, ep*X+x) <-> row (2*ep+eo)*128+p, col x."""
    r, x = m.shape
    return np.ascontiguousarray(
        m.reshape(NE // 2, 2, P, x).transpose(2, 1, 0, 3).reshape(P, 2, (NE // 2) * x)
    )


def _pack_ep(m):
    """[NE*128, X] -> [128, NE/2, 2, X] ep-major DoubleRow pairing:
    (p, ep, eo, x) <-> row (2*ep+eo)*128+p, col x."""
    r, x = m.shape
    return np.ascontiguousarray(
        m.reshape(NE // 2, 2, P, x).transpose(2, 0, 1, 3)
    )


def make_in_maps(q, k, v, Wq, Wk, Wv):
    """q,k,v: [4, 2048, 1024] f32; W*: [1024, 1024] f32 -> 8 per-core in_maps."""
    # merge the two score-side weights: scores = q (Wq^T Wk) k^T
    M = (Wq.T.astype(np.float64) @ Wk.astype(np.float64) * 32.0).astype(np.float32)
    wqT = _pack_ep(M.astype(nfp8))
    wvT = _pack(Wv.T.astype(nbf16))

    kk = np.arange(P)[:, None]
    qq = np.arange(P)[None, :]
    tri = np.where(qq >= kk, 0.0, MASK_NEG).astype(np.float32)
    mask_even = np.concatenate([tri, np.zeros((P, P), np.float32)], axis=1)
    mask_odd = np.concatenate([np.full((P, P), MASK_NEG, np.float32), tri], axis=1)

    in_maps = []
    for c in range(8):
        b, parity = c // 2, c % 2
        rows = np.concatenate(
            [np.arange(g * P, (g + 1) * P) for g in core_k_tiles(parity)]
        )
        in_maps.append({
            "qT": _pack_ep(
                q[b].T[:, parity * (SEQ // 2):(parity + 1) * (SEQ // 2)].astype(nfp8)
            ),
            "kT": _pack_pair(k[b][rows].T.astype(nfp8)),
            "vT": _pack(v[b][rows].T.astype(nbf16)),
            "wqT": wqT,
            "wvT": wvT,
            "mask": mask_even if parity == 0 else mask_odd,
        })
    return in_maps


def combine_outputs(outs):
    """outs: list of 8 [2048, 1024] partial arrays -> [4, 2048, 1024]."""
    res = np.empty((4, SEQ, H), np.float32)
    for b in range(4):
        res[b] = outs[2 * b].astype(np.float32) + outs[2 * b + 1].astype(np.float32)
    return res


_NC_CACHE = []


def kernel(q, k, v, Wq, Wk, Wv):
    """Full inputs in, full output out; 8-core TRN2 SPMD inside."""
    q = np.asarray(q, dtype=np.float32)
    k = np.asarray(k, dtype=np.float32)
    v = np.asarray(v, dtype=np.float32)
    Wq = np.asarray(Wq, dtype=np.float32)
    Wk = np.asarray(Wk, dtype=np.float32)
    Wv = np.asarray(Wv, dtype=np.float32)

    if not _NC_CACHE:
        _NC_CACHE.append(build_nc())
    nc = _NC_CACHE[0]

    in_maps = make_in_maps(q, k, v, Wq, Wk, Wv)
    res = run_bass_kernel_spmd(nc, in_maps, core_ids=list(range(8)))
    outs = [res.results[i]["out"] for i in range(8)]
    return combine_outputs(outs)



# revision 2
# speedup vs baseline: 1.1451x; 1.1451x over previous
"""Distributed causal attention (softmax over the QUERY axis) on 8 TRN2
NeuronCores, written in Bass/Tile.

Sharding: the reference normalizes softmax over the query axis (axis=1), so
each key-column's softmax is independent. We therefore shard the KEY axis:
core pair (2b, 2b+1) handles batch b, with even cores owning even 128-row
k-tiles and odd cores owning odd k-tiles. The interleaving makes the
causal-sparsity-aware SPMD instruction graph identical on all cores (the one
asymmetric bit - the diagonal mask - is per-core input data).

Both projection weights of the score path are merged on the host into
M = Wq^T Wk (x32 for fp8 range), and M is contracted into the K SIDE:
scores^T = (M k_c^T)^T q^T. Because each core owns its k rows, khT = M k_c^T
is computed entirely locally (half the FLOPs of a q-side projection), and the
score matmul contracts khT against RAW q - a host-packed input - so no
cross-core exchange, collective fence, or shared-HBM round-trip is needed at
all (the previous q-side variant lost ~10us to the CC-firmware fence chain).
Phase K' and the score matmuls run fully in fp8 DoubleRow (256-row
contractions; rel-err 1.66e-2 vs the 2e-2 gate, predicted exactly by a host
numpy simulation of the quantization chain); the exp applies the 1/(32*sqrt(H))
scale so all fp8 operands stay in e4m3's normal range. The v-projection and PV
stay bf16 (fp8 there adds ~3-4% error, over the gate). Inputs are host-packed
partition-major and loaded as large contiguous descriptors (HWDGE
descriptor-gen is ~600ns/instruction and ring bandwidth scales with
descriptor size), interleaved across the SP/ACT rings in exact consumption
order: M/k chunks (phase K'), full qT (phase D), v/Wv (phase B). The
per-k softmax reciprocal is computed in phase D and folded into vh as phase B
produces each chunk. The host sums the two partial PV outputs per batch pair.
"""

from contextlib import ExitStack

import numpy as np
import ml_dtypes

import concourse.bass as bass
from concourse import bacc
import concourse.tile as tile
import concourse.mybir as mybir
from concourse.bass_utils import run_bass_kernel_spmd
from concourse.tile import ScopedClock

BATCH = 4


def _fast_drain_and_barrier(self, tick_clock, wait_clock):
    """Tile kernel-tail with sem-only all-engine barriers (the default
    drain+butterfly pair costs ~8us); the explicit sync.drain with the global
    clock waits already covers all tracked work."""
    drain_inst = self.nc.sync.drain()
    wait_clock.add_sem_waits(
        drain_inst.ins, ScopedClock({None: tick_clock.global_clock})
    )
    self.nc.all_engine_barrier(sem_only=True)
    assert self.sems is not None
    popped = self.nc._tile_sem_poison_stack.pop()
    assert popped is self._sem_poison
    self.nc.clear_and_free_semaphores(list(self.sems.allocated().values()))
    self.nc.all_engine_barrier(sem_only=True)


tile.TileContext._drain_and_barrier = _fast_drain_and_barrier

P = 128
SEQ = 2048
E = 1024
H = 1024
KL = 1024          # k columns per core (16 tiles / 2 cores * 128)
NE = E // P        # 8
NH = H // P        # 8
NKS = KL // P      # 8 k slots per core
NQT = SEQ // P     # 16 q tiles
NB = 512           # matmul free-dim / psum bank
NEP = NE // 2      # 4 DoubleRow contraction pairs
MASK_NEG = -51200.0  # pre-exp-scale; exp applies 1/1024 -> effective -50
N_WARMUP = 12   # longer warmups regress: sustained dummy PE activity trips
                # the HAM/DVFS throttle before the real phases begin

BF16 = mybir.dt.bfloat16
FP8 = mybir.dt.float8e4
F32 = mybir.dt.float32
nbf16 = ml_dtypes.bfloat16
nfp8 = ml_dtypes.float8_e4m3


def slot_chunks(j):
    """(ext, [(off, width), ...]) q-chunks for score slot j (relative to 256j)."""
    ext = SEQ - 256 * j
    chunks = []
    off = 0
    if j % 2 == 1:
        chunks.append((0, 256))
        off = 256
    while off < ext:
        chunks.append((off, NB))
        off += NB
    return ext, chunks


def build_nc():
    nc = bacc.Bacc("TRN2", target_bir_lowering=False, debug=False, num_devices=8)
    # Inputs arrive pre-packed partition-major ep-interleaved for DoubleRow
    # ([128, NEP, 2, X]: partition p, (ep, ko, x) <-> logical row
    # (2*ep+ko)*128+p, col x) so each tensor loads with a few DMAs of large
    # contiguous descriptors.
    qT = nc.dram_tensor("qT", [P, NEP, 2, SEQ], FP8, kind="ExternalInput").ap()
    kT = nc.dram_tensor("kT", [P, NEP, 2, KL], FP8, kind="ExternalInput").ap()
    vT = nc.dram_tensor("vT", [P, NE * KL], BF16, kind="ExternalInput").ap()
    wqT = nc.dram_tensor("wqT", [P, NEP, 2, H], FP8, kind="ExternalInput").ap()
    wvT = nc.dram_tensor("wvT", [P, NE * H], BF16, kind="ExternalInput").ap()
    mask = nc.dram_tensor("mask", [P, 256], F32, kind="ExternalInput").ap()
    out = nc.dram_tensor("out", [SEQ, H], BF16, kind="ExternalOutput").ap()

    with tile.TileContext(nc) as tc, ExitStack() as ctx:
        wpool = ctx.enter_context(tc.tile_pool(name="w", bufs=1))
        ktv = ctx.enter_context(tc.tile_pool(name="ktv", bufs=1))
        qtp = ctx.enter_context(tc.tile_pool(name="qtp", bufs=1))
        khpool = ctx.enter_context(tc.tile_pool(name="kh", bufs=1))
        vhpool = ctx.enter_context(tc.tile_pool(name="vh", bufs=NKS))
        prpool = ctx.enter_context(tc.tile_pool(name="pr", bufs=1))
        smpool = ctx.enter_context(tc.tile_pool(name="sm", bufs=1))
        ostpool = ctx.enter_context(tc.tile_pool(name="ost", bufs=10))
        psum = ctx.enter_context(tc.tile_pool(name="ps", bufs=8, space="PSUM"))

        # ---- PE warmup: dummy matmuls with no input deps run during the
        # initial DMA wait, releasing the HAM clock throttle early ----
        wrm = smpool.tile([P, 256], BF16, tag="wrm", name="wrm")
        nc.vector.memset(wrm[:], 0.0)
        wps = psum.tile([P, 256], F32, tag="ps", name="wps")
        for i in range(N_WARMUP):
            nc.tensor.matmul(wps[:], lhsT=wrm[:, :P], rhs=wrm[:],
                             start=(i == 0), stop=(i == N_WARMUP - 1))

        # ---- mask ----
        msk = smpool.tile([P, 256], F32, tag="msk", name="msk")
        nc.sync.dma_start(msk[:], mask[:])

        # Alternate input DMAs between the SP and ACT HWDGE rings so
        # descriptor generation parallelizes.
        def in_dma(e, *args):
            return (nc.sync if e % 2 == 0 else nc.scalar).dma_start(*args)

        # ---- input DMAs in exact consumption order ----
        # Phase K' inputs first (M^T chunks interleaved with kT ep chunks so
        # the ep-inner accumulation can start as soon as the early eps land),
        # then full qT (phase D), then v/Wv (phase B).
        wq_sb = wpool.tile([P, NEP, 2, H], FP8, tag="wq", name="wq_sb")
        kt_sb = ktv.tile([P, NEP, 2, KL], FP8, tag="kt", name="kt_sb")
        in_dma(0, wq_sb[:, 0:2, :, :], wqT[:, 0:2, :, :])
        in_dma(1, kt_sb[:, 0:2, :, :], kT[:, 0:2, :, :])
        in_dma(0, kt_sb[:, 2:4, :, :], kT[:, 2:4, :, :])
        in_dma(1, wq_sb[:, 2:4, :, :], wqT[:, 2:4, :, :])

        qt_sb = qtp.tile([P, NEP, 2, SEQ], FP8, tag="qt", name="qt_sb")
        for c in range(4):
            in_dma(c, qt_sb[:, c:c + 1, :, :], qT[:, c:c + 1, :, :])

        wv_sb = wpool.tile([P, NE * H], BF16, tag="wv", name="wv_sb")
        vt_sb = ktv.tile([P, NE * KL], BF16, tag="vt", name="vt_sb")
        in_dma(0, vt_sb[:], vT[:])
        in_dma(1, wv_sb[:], wvT[:])

        # ---- phase K': khT[a, kl] = (M k_c^T), fp8 DoubleRow (256-row
        # contraction per matmul), computed entirely from this core's k rows.
        # Output packed [p, ep', ko', kl] (a = (2ep'+ko')*128+p) so phase D
        # can slice it as DoubleRow lhsT pairs directly. ----
        kh_sb = khpool.tile([P, NEP, 2, KL], FP8, tag="kh", name="kh_sb")
        for kb in range(KL // NB):
            pts = [psum.tile([P, NB], F32, tag="ps", name=f"pk{kb}_{at}")
                   for at in range(NH)]
            for ep in range(NEP):
                for at in range(NH):
                    nc.tensor.matmul(
                        pts[at][:],
                        lhsT=wq_sb[:, ep:ep + 1, :, at * P:(at + 1) * P]
                            .squeeze(1),
                        rhs=kt_sb[:, ep:ep + 1, :, kb * NB:(kb + 1) * NB]
                            .squeeze(1),
                        start=(ep == 0),
                        stop=(ep == NEP - 1),
                        perf_mode=mybir.MatmulPerfMode.DoubleRow,
                    )
            for at in range(NH):
                nc.vector.tensor_copy(
                    kh_sb[:, at // 2:at // 2 + 1, at % 2:at % 2 + 1,
                          kb * NB:(kb + 1) * NB].squeeze(1).squeeze(1),
                    pts[at][:])

        # ---- phase D: scoresT -> exp -> den -> rec (vh scaling happens in
        # phase B as each vh chunk is produced) ----
        pr_sb = []
        recs = []
        for j in range(NKS):
            ext, chunks = slot_chunks(j)
            q0 = 256 * j
            pr = prpool.tile([P, ext], BF16, tag=f"pr{j}", name=f"pr{j}")
            accs = smpool.tile([P, len(chunks)], F32, tag=f"acc{j}", name=f"acc{j}")
            for ci, (off, w) in enumerate(chunks):
                pt = psum.tile([P, NB], F32, tag="ps", name=f"sp{j}_{ci}")
                for ep in range(NEP):
                    nc.tensor.matmul(
                        pt[:, :w],
                        lhsT=kh_sb[:, ep:ep + 1, :, j * P:(j + 1) * P]
                            .squeeze(1),
                        rhs=qt_sb[:, ep:ep + 1, :, q0 + off:q0 + off + w]
                            .squeeze(1),
                        start=(ep == 0),
                        stop=(ep == NEP - 1),
                        perf_mode=mybir.MatmulPerfMode.DoubleRow,
                    )
                if off == 0:
                    nc.vector.tensor_add(pt[:, :256], pt[:, :256], msk[:])
                nc.scalar.activation(
                    pr[:, off:off + w],
                    pt[:, :w],
                    mybir.ActivationFunctionType.Exp,
                    scale=float(1.0 / (np.sqrt(H) * 32.0)),
                    accum_out=accs[:, ci:ci + 1],
                )
            den = smpool.tile([P, 1], F32, tag=f"den{j}", name=f"den{j}")
            nc.vector.tensor_reduce(
                den[:], accs[:], axis=mybir.AxisListType.X, op=mybir.AluOpType.add
            )
            rec = smpool.tile([P, 1], F32, tag=f"rec{j}", name=f"rec{j}")
            nc.vector.reciprocal(rec[:], den[:])
            pr_sb.append(pr)
            recs.append(rec)

        # ---- phase B: vh[kl, h] (e-outer, 8 concurrent banks); each chunk is
        # scaled by the softmax reciprocal (from phase D) as it is copied ----
        vh_sb = [vhpool.tile([P, H], BF16, tag="vh", name=f"vh{j}")
                 for j in range(NKS)]
        for hb in range(H // NB):
            pts = [psum.tile([P, NB], F32, tag="ps", name=f"pp_vh{j}_{hb}")
                   for j in range(NKS)]
            for e in range(NE):
                for j in range(NKS):
                    nc.tensor.matmul(
                        pts[j][:],
                        lhsT=vt_sb[:, e * KL + j * P:e * KL + (j + 1) * P],
                        rhs=wv_sb[:, e * H + hb * NB:e * H + (hb + 1) * NB],
                        start=(e == 0),
                        stop=(e == NE - 1),
                    )
            for j in range(NKS):
                sl = vh_sb[j][:, hb * NB:(hb + 1) * NB]
                nc.vector.tensor_copy(sl, pts[j][:])
                nc.vector.tensor_scalar_mul(sl, sl, recs[j][:])

        # ---- phase E: PV + output ----
        # Interleave big-t (many accumulation MMs) and small-t (few) groups so
        # the DVE copy + out-DMA drain keeps pace with PSUM-bank production.
        pv_order = []
        lo, hi = 0, NQT - 1
        while lo <= hi:
            pv_order.append(hi)
            if lo < hi:
                pv_order.append(lo)
            hi -= 1
            lo += 1
        for t in pv_order:
            jmax = t // 2
            for hb in range(H // NB):
                pt = psum.tile([P, NB], F32, tag="ps", name=f"pv{t}_{hb}")
                for j in range(jmax + 1):
                    off = t * P - 256 * j
                    nc.tensor.matmul(
                        pt[:],
                        lhsT=pr_sb[j][:, off:off + P],
                        rhs=vh_sb[j][:, hb * NB:(hb + 1) * NB],
                        start=(j == 0),
                        stop=(j == jmax),
                    )
                ot = ostpool.tile([P, NB], BF16, tag="ost", name=f"ot{t}_{hb}")
                if t == pv_order[-1]:
                    # final tile: halve copy+DMA so the drain tail pipelines
                    for ha in range(2):
                        sl = slice(ha * (NB // 2), (ha + 1) * (NB // 2))
                        nc.vector.tensor_copy(ot[:, sl], pt[:, sl])
                        eng2 = nc.sync if (hb + ha) % 2 == 0 else nc.scalar
                        eng2.dma_start(
                            out[t * P:(t + 1) * P,
                                hb * NB + ha * (NB // 2):
                                hb * NB + (ha + 1) * (NB // 2)],
                            ot[:, sl],
                        )
                    continue
                nc.vector.tensor_copy(ot[:], pt[:])
                n_done = len([x for x in pv_order[:pv_order.index(t) + 1]])
                eng = nc.gpsimd if n_done <= NQT - 4 else (
                    nc.sync if hb == 0 else nc.scalar)
                eng.dma_start(
                    out[t * P:(t + 1) * P, hb * NB:(hb + 1) * NB], ot[:]
                )

    nc.compile()
    return nc


# ---------------- host-side prep ----------------

def core_k_tiles(parity):
    return list(range(parity, 16, 2))


def _pack(m):
    """[NE*128, X] -> [128, NE*X]: partition-major so DMA descriptors are
    large and contiguous (row e*128+p, col x) -> (p, e*X+x)."""
    r, x = m.shape
    return np.ascontiguousarray(
        m.reshape(NE, P, x).transpose(1, 0, 2).reshape(P, NE * x)
    )


def _pack_ep(m):
    """[NE*128, X] -> [128, NE/2, 2, X] ep-major DoubleRow pairing:
    (p, ep, eo, x) <-> row (2*ep+eo)*128+p, col x."""
    r, x = m.shape
    return np.ascontiguousarray(
        m.reshape(NE // 2, 2, P, x).transpose(2, 0, 1, 3)
    )


def make_in_maps(q, k, v, Wq, Wk, Wv):
    """q,k,v: [4, 2048, 1024] f32; W*: [1024, 1024] f32 -> 8 per-core in_maps."""
    # merge the two score-side weights: scores = q (Wq^T Wk) k^T; M is
    # contracted into the k side on-device (khT = M k_c^T), so ship M^T
    # (contraction dim c on partitions).
    M = (Wq.T.astype(np.float64) @ Wk.astype(np.float64) * 32.0).astype(np.float32)
    wqT = _pack_ep(np.ascontiguousarray(M.T).astype(nfp8))
    wvT = _pack(Wv.T.astype(nbf16))

    kk = np.arange(P)[:, None]
    qq = np.arange(P)[None, :]
    tri = np.where(qq >= kk, 0.0, MASK_NEG).astype(np.float32)
    mask_even = np.concatenate([tri, np.zeros((P, P), np.float32)], axis=1)
    mask_odd = np.concatenate([np.full((P, P), MASK_NEG, np.float32), tri], axis=1)

    in_maps = []
    for c in range(8):
        b, parity = c // 2, c % 2
        rows = np.concatenate(
            [np.arange(g * P, (g + 1) * P) for g in core_k_tiles(parity)]
        )
        in_maps.append({
            "qT": _pack_ep(q[b].T.astype(nfp8)),
            "kT": _pack_ep(np.ascontiguousarray(k[b][rows].T).astype(nfp8)),
            "vT": _pack(v[b][rows].T.astype(nbf16)),
            "wqT": wqT,
            "wvT": wvT,
            "mask": mask_even if parity == 0 else mask_odd,
        })
    return in_maps


def combine_outputs(outs):
    """outs: list of 8 [2048, 1024] partial arrays -> [4, 2048, 1024]."""
    res = np.empty((4, SEQ, H), np.float32)
    for b in range(4):
        res[b] = outs[2 * b].astype(np.float32) + outs[2 * b + 1].astype(np.float32)
    return res


_NC_CACHE = []


def kernel(q, k, v, Wq, Wk, Wv):
    """Full inputs in, full output out; 8-core TRN2 SPMD inside."""
    q = np.asarray(q, dtype=np.float32)
    k = np.asarray(k, dtype=np.float32)
    v = np.asarray(v, dtype=np.float32)
    Wq = np.asarray(Wq, dtype=np.float32)
    Wk = np.asarray(Wk, dtype=np.float32)
    Wv = np.asarray(Wv, dtype=np.float32)

    if not _NC_CACHE:
        _NC_CACHE.append(build_nc())
    nc = _NC_CACHE[0]

    in_maps = make_in_maps(q, k, v, Wq, Wk, Wv)
    res = run_bass_kernel_spmd(nc, in_maps, core_ids=list(range(8)))
    outs = [res.results[i]["out"] for i in range(8)]
    return combine_outputs(outs)


# revision 5
# speedup vs baseline: 1.1528x; 1.0067x over previous
"""Distributed causal attention (softmax over the QUERY axis) on 8 TRN2
NeuronCores, written in Bass/Tile.

Sharding: the reference normalizes softmax over the query axis (axis=1), so
each key-column's softmax is independent. We therefore shard the KEY axis:
core pair (2b, 2b+1) handles batch b, with even cores owning even 128-row
k-tiles and odd cores owning odd k-tiles. The interleaving makes the
causal-sparsity-aware SPMD instruction graph identical on all cores (the one
asymmetric bit - the diagonal mask - is per-core input data).

Both projection weights of the score path are merged on the host into
M = Wq^T Wk (x32 for fp8 range), and M is contracted into the K SIDE:
scores^T = (M k_c^T)^T q^T. Because each core owns its k rows, khT = M k_c^T
is computed entirely locally (half the FLOPs of a q-side projection), and the
score matmul contracts khT against RAW q - a host-packed input - so no
cross-core exchange, collective fence, or shared-HBM round-trip is needed at
all (the previous q-side variant lost ~10us to the CC-firmware fence chain).
Phase K' and the score matmuls run fully in fp8 DoubleRow (256-row
contractions; rel-err 1.66e-2 vs the 2e-2 gate, predicted exactly by a host
numpy simulation of the quantization chain); the exp applies the 1/(32*sqrt(H))
scale so all fp8 operands stay in e4m3's normal range. The v-projection and PV
stay bf16 (fp8 there adds ~3-4% error, over the gate). Inputs are host-packed
partition-major and loaded as large contiguous descriptors (HWDGE
descriptor-gen is ~600ns/instruction and ring bandwidth scales with
descriptor size), interleaved across the SP/ACT rings in exact consumption
order: M/k chunks (phase K'), full qT (phase D), v/Wv (phase B). The
per-k softmax reciprocal is computed in phase D and folded into vh as phase B
produces each chunk. The host sums the two partial PV outputs per batch pair.
"""

from contextlib import ExitStack

import numpy as np
import ml_dtypes

import concourse.bass as bass
from concourse import bacc
import concourse.tile as tile
import concourse.mybir as mybir
from concourse.bass_utils import run_bass_kernel_spmd
from concourse.tile import ScopedClock

BATCH = 4


def _fast_drain_and_barrier(self, tick_clock, wait_clock):
    """Tile kernel-tail with sem-only all-engine barriers (the default
    drain+butterfly pair costs ~8us); the explicit sync.drain with the global
    clock waits already covers all tracked work."""
    drain_inst = self.nc.sync.drain()
    wait_clock.add_sem_waits(
        drain_inst.ins, ScopedClock({None: tick_clock.global_clock})
    )
    self.nc.all_engine_barrier(sem_only=True)
    assert self.sems is not None
    popped = self.nc._tile_sem_poison_stack.pop()
    assert popped is self._sem_poison
    self.nc.clear_and_free_semaphores(list(self.sems.allocated().values()))
    self.nc.all_engine_barrier(sem_only=True)


tile.TileContext._drain_and_barrier = _fast_drain_and_barrier

P = 128
SEQ = 2048
E = 1024
H = 1024
KL = 1024          # k columns per core (16 tiles / 2 cores * 128)
NE = E // P        # 8
NH = H // P        # 8
NKS = KL // P      # 8 k slots per core
NQT = SEQ // P     # 16 q tiles
NB = 512           # matmul free-dim / psum bank
NEP = NE // 2      # 4 DoubleRow contraction pairs
MASK_NEG = -51200.0  # pre-exp-scale; exp applies 1/1024 -> effective -50
N_WARMUP = 14   # sized so the warmup hands off seamlessly into phase K'
                # (any PE idle gap resets the HAM un-throttle window)

BF16 = mybir.dt.bfloat16
FP8 = mybir.dt.float8e4
F32 = mybir.dt.float32
nbf16 = ml_dtypes.bfloat16
nfp8 = ml_dtypes.float8_e4m3


def slot_chunks(j):
    """(ext, [(off, width), ...]) q-chunks for score slot j (relative to 256j)."""
    ext = SEQ - 256 * j
    chunks = []
    off = 0
    if j % 2 == 1:
        chunks.append((0, 256))
        off = 256
    while off < ext:
        chunks.append((off, NB))
        off += NB
    return ext, chunks


def build_nc():
    nc = bacc.Bacc("TRN2", target_bir_lowering=False, debug=False, num_devices=8)
    # Inputs arrive pre-packed partition-major ep-interleaved for DoubleRow
    # ([128, NEP, 2, X]: partition p, (ep, ko, x) <-> logical row
    # (2*ep+ko)*128+p, col x) so each tensor loads with a few DMAs of large
    # contiguous descriptors.
    qT = nc.dram_tensor("qT", [P, NEP, 2, SEQ], FP8, kind="ExternalInput").ap()
    kT = nc.dram_tensor("kT", [P, NEP, 2, KL], FP8, kind="ExternalInput").ap()
    vT = nc.dram_tensor("vT", [P, NE * KL], BF16, kind="ExternalInput").ap()
    wqT = nc.dram_tensor("wqT", [P, NEP, 2, H], FP8, kind="ExternalInput").ap()
    wvT = nc.dram_tensor("wvT", [P, NE * H], BF16, kind="ExternalInput").ap()
    mask = nc.dram_tensor("mask", [P, 256], F32, kind="ExternalInput").ap()
    out = nc.dram_tensor("out", [SEQ, H], BF16, kind="ExternalOutput").ap()

    with tile.TileContext(nc) as tc, ExitStack() as ctx:
        wpool = ctx.enter_context(tc.tile_pool(name="w", bufs=1))
        ktv = ctx.enter_context(tc.tile_pool(name="ktv", bufs=1))
        qtp = ctx.enter_context(tc.tile_pool(name="qtp", bufs=1))
        khpool = ctx.enter_context(tc.tile_pool(name="kh", bufs=1))
        vhpool = ctx.enter_context(tc.tile_pool(name="vh", bufs=NKS))
        prpool = ctx.enter_context(tc.tile_pool(name="pr", bufs=1))
        smpool = ctx.enter_context(tc.tile_pool(name="sm", bufs=1))
        ostpool = ctx.enter_context(tc.tile_pool(name="ost", bufs=10))
        psum = ctx.enter_context(tc.tile_pool(name="ps", bufs=8, space="PSUM"))

        # ---- PE warmup: dummy matmuls with no input deps run during the
        # initial DMA wait, releasing the HAM clock throttle early ----
        wrm = smpool.tile([P, 256], BF16, tag="wrm", name="wrm")
        nc.vector.memset(wrm[:], 0.0)
        wps = psum.tile([P, 256], F32, tag="ps", name="wps")
        for i in range(N_WARMUP):
            nc.tensor.matmul(wps[:], lhsT=wrm[:, :P], rhs=wrm[:],
                             start=(i == 0), stop=(i == N_WARMUP - 1))

        # Alternate input DMAs between the SP and ACT HWDGE rings so
        # descriptor generation parallelizes.
        def in_dma(e, *args):
            return (nc.sync if e % 2 == 0 else nc.scalar).dma_start(*args)

        # ---- input DMAs in exact consumption order ----
        # Phase K' inputs first as single-ep chunks, one M^T and one kT chunk
        # per ring per step, so ep0 lands (and phase K' starts) as early as
        # the cold uncore allows; then full qT (phase D), the mask, and
        # v/Wv (phase B).
        wq_sb = wpool.tile([P, NEP, 2, H], FP8, tag="wq", name="wq_sb")
        kt_sb = ktv.tile([P, NEP, 2, KL], FP8, tag="kt", name="kt_sb")
        for ep in range(NEP):
            in_dma(ep, wq_sb[:, ep:ep + 1, :, :], wqT[:, ep:ep + 1, :, :])
            in_dma(ep + 1, kt_sb[:, ep:ep + 1, :, :], kT[:, ep:ep + 1, :, :])

        qt_sb = qtp.tile([P, NEP, 2, SEQ], FP8, tag="qt", name="qt_sb")
        for c in range(4):
            in_dma(c, qt_sb[:, c:c + 1, :, :], qT[:, c:c + 1, :, :])

        msk = smpool.tile([P, 256], F32, tag="msk", name="msk")
        nc.sync.dma_start(msk[:], mask[:])

        wv_sb = wpool.tile([P, NE * H], BF16, tag="wv", name="wv_sb")
        vt_sb = ktv.tile([P, NE * KL], BF16, tag="vt", name="vt_sb")
        in_dma(0, vt_sb[:], vT[:])
        in_dma(1, wv_sb[:], wvT[:])

        # ---- phase K': khT[a, kl] = (M k_c^T), fp8 DoubleRow (256-row
        # contraction per matmul), computed entirely from this core's k rows.
        # Output packed [p, ep', ko', kl] (a = (2ep'+ko')*128+p) so phase D
        # can slice it as DoubleRow lhsT pairs directly. ----
        kh_sb = khpool.tile([P, NEP, 2, KL], FP8, tag="kh", name="kh_sb")
        for kb in range(KL // NB):
            pts = [psum.tile([P, NB], F32, tag="ps", name=f"pk{kb}_{at}")
                   for at in range(NH)]
            for ep in range(NEP):
                for at in range(NH):
                    nc.tensor.matmul(
                        pts[at][:],
                        lhsT=wq_sb[:, ep:ep + 1, :, at * P:(at + 1) * P]
                            .squeeze(1),
                        rhs=kt_sb[:, ep:ep + 1, :, kb * NB:(kb + 1) * NB]
                            .squeeze(1),
                        start=(ep == 0),
                        stop=(ep == NEP - 1),
                        perf_mode=mybir.MatmulPerfMode.DoubleRow,
                    )
            for at in range(NH):
                nc.vector.tensor_copy(
                    kh_sb[:, at // 2:at // 2 + 1, at % 2:at % 2 + 1,
                          kb * NB:(kb + 1) * NB].squeeze(1).squeeze(1),
                    pts[at][:])

        # ---- phase D: scoresT -> exp -> den -> rec (vh scaling happens in
        # phase B as each vh chunk is produced) ----
        pr_sb = []
        recs = []
        for j in range(NKS):
            ext, chunks = slot_chunks(j)
            q0 = 256 * j
            pr = prpool.tile([P, ext], BF16, tag=f"pr{j}", name=f"pr{j}")
            accs = smpool.tile([P, len(chunks)], F32, tag=f"acc{j}", name=f"acc{j}")
            for ci, (off, w) in enumerate(chunks):
                pt = psum.tile([P, NB], F32, tag="ps", name=f"sp{j}_{ci}")
                for ep in range(NEP):
                    nc.tensor.matmul(
                        pt[:, :w],
                        lhsT=kh_sb[:, ep:ep + 1, :, j * P:(j + 1) * P]
                            .squeeze(1),
                        rhs=qt_sb[:, ep:ep + 1, :, q0 + off:q0 + off + w]
                            .squeeze(1),
                        start=(ep == 0),
                        stop=(ep == NEP - 1),
                        perf_mode=mybir.MatmulPerfMode.DoubleRow,
                    )
                if off == 0:
                    nc.vector.tensor_add(pt[:, :256], pt[:, :256], msk[:])
                nc.scalar.activation(
                    pr[:, off:off + w],
                    pt[:, :w],
                    mybir.ActivationFunctionType.Exp,
                    scale=float(1.0 / (np.sqrt(H) * 32.0)),
                    accum_out=accs[:, ci:ci + 1],
                )
            den = smpool.tile([P, 1], F32, tag=f"den{j}", name=f"den{j}")
            nc.vector.tensor_reduce(
                den[:], accs[:], axis=mybir.AxisListType.X, op=mybir.AluOpType.add
            )
            rec = smpool.tile([P, 1], F32, tag=f"rec{j}", name=f"rec{j}")
            nc.vector.reciprocal(rec[:], den[:])
            pr_sb.append(pr)
            recs.append(rec)

        # ---- phase B: vh[kl, h] (e-outer, 8 concurrent banks); each chunk is
        # scaled by the softmax reciprocal (from phase D) as it is copied ----
        vh_sb = [vhpool.tile([P, H], BF16, tag="vh", name=f"vh{j}")
                 for j in range(NKS)]
        for hb in range(H // NB):
            pts = [psum.tile([P, NB], F32, tag="ps", name=f"pp_vh{j}_{hb}")
                   for j in range(NKS)]
            for e in range(NE):
                for j in range(NKS):
                    nc.tensor.matmul(
                        pts[j][:],
                        lhsT=vt_sb[:, e * KL + j * P:e * KL + (j + 1) * P],
                        rhs=wv_sb[:, e * H + hb * NB:e * H + (hb + 1) * NB],
                        start=(e == 0),
                        stop=(e == NE - 1),
                    )
            for j in range(NKS):
                sl = vh_sb[j][:, hb * NB:(hb + 1) * NB]
                nc.vector.tensor_copy(sl, pts[j][:])
                nc.vector.tensor_scalar_mul(sl, sl, recs[j][:])

        # ---- phase E: PV + output ----
        # Interleave big-t (many accumulation MMs) and small-t (few) groups so
        # the DVE copy + out-DMA drain keeps pace with PSUM-bank production.
        # t=1,0 (fewest accumulation MMs) go last so the post-last-matmul
        # copy+DMA drain is as short as possible.
        pv_order = []
        lo, hi = 2, NQT - 1
        while lo <= hi:
            pv_order.append(hi)
            if lo < hi:
                pv_order.append(lo)
            hi -= 1
            lo += 1
        pv_order += [1, 0]
        for t in pv_order:
            jmax = t // 2
            for hb in range(H // NB):
                pt = psum.tile([P, NB], F32, tag="ps", name=f"pv{t}_{hb}")
                for j in range(jmax + 1):
                    off = t * P - 256 * j
                    nc.tensor.matmul(
                        pt[:],
                        lhsT=pr_sb[j][:, off:off + P],
                        rhs=vh_sb[j][:, hb * NB:(hb + 1) * NB],
                        start=(j == 0),
                        stop=(j == jmax),
                    )
                ot = ostpool.tile([P, NB], BF16, tag="ost", name=f"ot{t}_{hb}")
                if t == pv_order[-1]:
                    # final tile: halve copy+DMA so the drain tail pipelines
                    for ha in range(2):
                        sl = slice(ha * (NB // 2), (ha + 1) * (NB // 2))
                        nc.vector.tensor_copy(ot[:, sl], pt[:, sl])
                        eng2 = nc.sync if (hb + ha) % 2 == 0 else nc.scalar
                        eng2.dma_start(
                            out[t * P:(t + 1) * P,
                                hb * NB + ha * (NB // 2):
                                hb * NB + (ha + 1) * (NB // 2)],
                            ot[:, sl],
                        )
                    continue
                nc.vector.tensor_copy(ot[:], pt[:])
                n_done = len([x for x in pv_order[:pv_order.index(t) + 1]])
                eng = nc.gpsimd if n_done <= NQT - 4 else (
                    nc.sync if hb == 0 else nc.scalar)
                eng.dma_start(
                    out[t * P:(t + 1) * P, hb * NB:(hb + 1) * NB], ot[:]
                )

    nc.compile()
    return nc


# ---------------- host-side prep ----------------

def core_k_tiles(parity):
    return list(range(parity, 16, 2))


def _pack(m):
    """[NE*128, X] -> [128, NE*X]: partition-major so DMA descriptors are
    large and contiguous (row e*128+p, col x) -> (p, e*X+x)."""
    r, x = m.shape
    return np.ascontiguousarray(
        m.reshape(NE, P, x).transpose(1, 0, 2).reshape(P, NE * x)
    )


def _pack_ep(m):
    """[NE*128, X] -> [128, NE/2, 2, X] ep-major DoubleRow pairing:
    (p, ep, eo, x) <-> row (2*ep+eo)*128+p, col x."""
    r, x = m.shape
    return np.ascontiguousarray(
        m.reshape(NE // 2, 2, P, x).transpose(2, 0, 1, 3)
    )


def make_in_maps(q, k, v, Wq, Wk, Wv):
    """q,k,v: [4, 2048, 1024] f32; W*: [1024, 1024] f32 -> 8 per-core in_maps."""
    # merge the two score-side weights: scores = q (Wq^T Wk) k^T; M is
    # contracted into the k side on-device (khT = M k_c^T), so ship M^T
    # (contraction dim c on partitions).
    M = (Wq.T.astype(np.float64) @ Wk.astype(np.float64) * 32.0).astype(np.float32)
    wqT = _pack_ep(np.ascontiguousarray(M.T).astype(nfp8))
    wvT = _pack(Wv.T.astype(nbf16))

    kk = np.arange(P)[:, None]
    qq = np.arange(P)[None, :]
    tri = np.where(qq >= kk, 0.0, MASK_NEG).astype(np.float32)
    mask_even = np.concatenate([tri, np.zeros((P, P), np.float32)], axis=1)
    mask_odd = np.concatenate([np.full((P, P), MASK_NEG, np.float32), tri], axis=1)

    in_maps = []
    for c in range(8):
        b, parity = c // 2, c % 2
        rows = np.concatenate(
            [np.arange(g * P, (g + 1) * P) for g in core_k_tiles(parity)]
        )
        in_maps.append({
            "qT": _pack_ep(q[b].T.astype(nfp8)),
            "kT": _pack_ep(np.ascontiguousarray(k[b][rows].T).astype(nfp8)),
            "vT": _pack(v[b][rows].T.astype(nbf16)),
            "wqT": wqT,
            "wvT": wvT,
            "mask": mask_even if parity == 0 else mask_odd,
        })
    return in_maps


def combine_outputs(outs):
    """outs: list of 8 [2048, 1024] partial arrays -> [4, 2048, 1024]."""
    res = np.empty((4, SEQ, H), np.float32)
    for b in range(4):
        res[b] = outs[2 * b].astype(np.float32) + outs[2 * b + 1].astype(np.float32)
    return res


_NC_CACHE = []


def kernel(q, k, v, Wq, Wk, Wv):
    """Full inputs in, full output out; 8-core TRN2 SPMD inside."""
    q = np.asarray(q, dtype=np.float32)
    k = np.asarray(k, dtype=np.float32)
    v = np.asarray(v, dtype=np.float32)
    Wq = np.asarray(Wq, dtype=np.float32)
    Wk = np.asarray(Wk, dtype=np.float32)
    Wv = np.asarray(Wv, dtype=np.float32)

    if not _NC_CACHE:
        _NC_CACHE.append(build_nc())
    nc = _NC_CACHE[0]

    in_maps = make_in_maps(q, k, v, Wq, Wk, Wv)
    res = run_bass_kernel_spmd(nc, in_maps, core_ids=list(range(8)))
    outs = [res.results[i]["out"] for i in range(8)]
    return combine_outputs(outs)


# revision 6
# speedup vs baseline: 1.1641x; 1.0098x over previous
"""Distributed causal attention (softmax over the QUERY axis) on 8 TRN2
NeuronCores, written in Bass/Tile.

Sharding: the reference normalizes softmax over the query axis (axis=1), so
each key-column's softmax is independent. We therefore shard the KEY axis:
core pair (2b, 2b+1) handles batch b, with even cores owning even 128-row
k-tiles and odd cores owning odd k-tiles. The interleaving makes the
causal-sparsity-aware SPMD instruction graph identical on all cores (the one
asymmetric bit - the diagonal mask - is per-core input data).

Both projection weights of the score path are merged on the host into
M = Wq^T Wk (x32 for fp8 range), and M is contracted into the K SIDE:
scores^T = (M k_c^T)^T q^T. Because each core owns its k rows, khT = M k_c^T
is computed entirely locally (half the FLOPs of a q-side projection), and the
score matmul contracts khT against RAW q - a host-packed input - so no
cross-core exchange, collective fence, or shared-HBM round-trip is needed at
all (the previous q-side variant lost ~10us to the CC-firmware fence chain).
Phase K' and the score matmuls run fully in fp8 DoubleRow (256-row
contractions; rel-err 1.66e-2 vs the 2e-2 gate, predicted exactly by a host
numpy simulation of the quantization chain); the exp applies the 1/(32*sqrt(H))
scale so all fp8 operands stay in e4m3's normal range. The v-projection and PV
stay bf16 (fp8 there adds ~3-4% error, over the gate). Inputs are host-packed
partition-major and loaded as large contiguous descriptors (HWDGE
descriptor-gen is ~600ns/instruction and ring bandwidth scales with
descriptor size), interleaved across the SP/ACT rings in exact consumption
order: M/k chunks (phase K'), full qT (phase D), v/Wv (phase B). The
per-k softmax reciprocal is computed in phase D and folded into vh as phase B
produces each chunk. The host sums the two partial PV outputs per batch pair.
"""

from contextlib import ExitStack

import numpy as np
import ml_dtypes

import concourse.bass as bass
from concourse import bacc
import concourse.tile as tile
import concourse.mybir as mybir
from concourse.bass_utils import run_bass_kernel_spmd
from concourse.tile import ScopedClock

BATCH = 4


def _fast_drain_and_barrier(self, tick_clock, wait_clock):
    """Tile kernel-tail with sem-only all-engine barriers (the default
    drain+butterfly pair costs ~8us); the explicit sync.drain with the global
    clock waits already covers all tracked work."""
    drain_inst = self.nc.sync.drain()
    wait_clock.add_sem_waits(
        drain_inst.ins, ScopedClock({None: tick_clock.global_clock})
    )
    self.nc.all_engine_barrier(sem_only=True)
    assert self.sems is not None
    popped = self.nc._tile_sem_poison_stack.pop()
    assert popped is self._sem_poison
    self.nc.clear_and_free_semaphores(list(self.sems.allocated().values()))
    self.nc.all_engine_barrier(sem_only=True)


tile.TileContext._drain_and_barrier = _fast_drain_and_barrier

P = 128
SEQ = 2048
E = 1024
H = 1024
KL = 1024          # k columns per core (16 tiles / 2 cores * 128)
NE = E // P        # 8
NH = H // P        # 8
NKS = KL // P      # 8 k slots per core
NQT = SEQ // P     # 16 q tiles
NB = 512           # matmul free-dim / psum bank
NEP = NE // 2      # 4 DoubleRow contraction pairs
MASK_NEG = -51200.0  # pre-exp-scale; exp applies 1/1024 -> effective -50
N_WARMUP = 14   # sized so the warmup hands off seamlessly into phase K'
                # (any PE idle gap resets the HAM un-throttle window)

BF16 = mybir.dt.bfloat16
FP8 = mybir.dt.float8e4
F32 = mybir.dt.float32
nbf16 = ml_dtypes.bfloat16
nfp8 = ml_dtypes.float8_e4m3


def slot_chunks(j):
    """(ext, [(off, width), ...]) q-chunks for score slot j (relative to 256j)."""
    ext = SEQ - 256 * j
    chunks = []
    off = 0
    if j % 2 == 1:
        chunks.append((0, 256))
        off = 256
    while off < ext:
        chunks.append((off, NB))
        off += NB
    return ext, chunks


def build_nc():
    nc = bacc.Bacc("TRN2", target_bir_lowering=False, debug=False, num_devices=8)
    # Inputs arrive pre-packed partition-major ep-interleaved for DoubleRow
    # ([128, NEP, 2, X]: partition p, (ep, ko, x) <-> logical row
    # (2*ep+ko)*128+p, col x) so each tensor loads with a few DMAs of large
    # contiguous descriptors.
    qT = nc.dram_tensor("qT", [P, NEP, 2, SEQ], FP8, kind="ExternalInput").ap()
    kT = nc.dram_tensor("kT", [P, NEP, 2, KL], FP8, kind="ExternalInput").ap()
    vT = nc.dram_tensor("vT", [P, NE * KL], BF16, kind="ExternalInput").ap()
    wqT = nc.dram_tensor("wqT", [P, NEP, 2, H], FP8, kind="ExternalInput").ap()
    wvT = nc.dram_tensor("wvT", [P, NE * H], BF16, kind="ExternalInput").ap()
    mask = nc.dram_tensor("mask", [P, 256], F32, kind="ExternalInput").ap()
    out = nc.dram_tensor("out", [SEQ, H], BF16, kind="ExternalOutput").ap()

    with tile.TileContext(nc) as tc, ExitStack() as ctx:
        wpool = ctx.enter_context(tc.tile_pool(name="w", bufs=1))
        ktv = ctx.enter_context(tc.tile_pool(name="ktv", bufs=1))
        qtp = ctx.enter_context(tc.tile_pool(name="qtp", bufs=1))
        khpool = ctx.enter_context(tc.tile_pool(name="kh", bufs=1))
        vhpool = ctx.enter_context(tc.tile_pool(name="vh", bufs=NKS))
        prpool = ctx.enter_context(tc.tile_pool(name="pr", bufs=1))
        smpool = ctx.enter_context(tc.tile_pool(name="sm", bufs=1))
        ostpool = ctx.enter_context(tc.tile_pool(name="ost", bufs=10))
        psum = ctx.enter_context(tc.tile_pool(name="ps", bufs=8, space="PSUM"))

        # ---- PE warmup: dummy matmuls with no input deps run during the
        # initial DMA wait, releasing the HAM clock throttle early ----
        wrm = smpool.tile([P, 256], BF16, tag="wrm", name="wrm")
        nc.vector.memset(wrm[:], 0.0)
        wps = psum.tile([P, 256], F32, tag="ps", name="wps")
        for i in range(N_WARMUP):
            nc.tensor.matmul(wps[:], lhsT=wrm[:, :P], rhs=wrm[:],
                             start=(i == 0), stop=(i == N_WARMUP - 1))

        # Alternate input DMAs between the SP and ACT HWDGE rings so
        # descriptor generation parallelizes.
        def in_dma(e, *args):
            return (nc.sync if e % 2 == 0 else nc.scalar).dma_start(*args)

        # ---- input DMAs in exact consumption order ----
        # Phase K' inputs first as single-ep chunks, one M^T and one kT chunk
        # per ring per step, so ep0 lands (and phase K' starts) as early as
        # the cold uncore allows; then full qT (phase D), the mask, and
        # v/Wv (phase B).
        wq_sb = wpool.tile([P, NEP, 2, H], FP8, tag="wq", name="wq_sb")
        kt_sb = ktv.tile([P, NEP, 2, KL], FP8, tag="kt", name="kt_sb")
        for ep in range(NEP):
            in_dma(ep, wq_sb[:, ep:ep + 1, :, :], wqT[:, ep:ep + 1, :, :])
            in_dma(ep + 1, kt_sb[:, ep:ep + 1, :, :], kT[:, ep:ep + 1, :, :])

        qt_sb = qtp.tile([P, NEP, 2, SEQ], FP8, tag="qt", name="qt_sb")
        for c in range(4):
            in_dma(c, qt_sb[:, c:c + 1, :, :], qT[:, c:c + 1, :, :])

        msk = smpool.tile([P, 256], F32, tag="msk", name="msk")
        nc.sync.dma_start(msk[:], mask[:])

        wv_sb = wpool.tile([P, NE * H], BF16, tag="wv", name="wv_sb")
        vt_sb = ktv.tile([P, NE * KL], BF16, tag="vt", name="vt_sb")
        in_dma(0, vt_sb[:], vT[:])
        in_dma(1, wv_sb[:], wvT[:])

        # ---- phase K': khT[a, kl] = (M k_c^T), fp8 DoubleRow (256-row
        # contraction per matmul), computed entirely from this core's k rows.
        # Output packed [p, ep', ko', kl] (a = (2ep'+ko')*128+p) so phase D
        # can slice it as DoubleRow lhsT pairs directly. ----
        kh_sb = khpool.tile([P, NEP, 2, KL], FP8, tag="kh", name="kh_sb")
        for kb in range(KL // NB):
            pts = [psum.tile([P, NB], F32, tag="ps", name=f"pk{kb}_{at}")
                   for at in range(NH)]
            for ep in range(NEP):
                for at in range(NH):
                    nc.tensor.matmul(
                        pts[at][:],
                        lhsT=wq_sb[:, ep:ep + 1, :, at * P:(at + 1) * P]
                            .squeeze(1),
                        rhs=kt_sb[:, ep:ep + 1, :, kb * NB:(kb + 1) * NB]
                            .squeeze(1),
                        start=(ep == 0),
                        stop=(ep == NEP - 1),
                        perf_mode=mybir.MatmulPerfMode.DoubleRow,
                    )
            for at in range(NH):
                nc.vector.tensor_copy(
                    kh_sb[:, at // 2:at // 2 + 1, at % 2:at % 2 + 1,
                          kb * NB:(kb + 1) * NB].squeeze(1).squeeze(1),
                    pts[at][:])

        # ---- phase D: scoresT -> exp -> den -> rec (vh scaling happens in
        # phase B as each vh chunk is produced) ----
        pr_sb = []
        recs = []
        for j in range(NKS):
            ext, chunks = slot_chunks(j)
            q0 = 256 * j
            pr = prpool.tile([P, ext], BF16, tag=f"pr{j}", name=f"pr{j}")
            accs = smpool.tile([P, len(chunks)], F32, tag=f"acc{j}", name=f"acc{j}")
            for ci, (off, w) in enumerate(chunks):
                pt = psum.tile([P, NB], F32, tag="ps", name=f"sp{j}_{ci}")
                for ep in range(NEP):
                    nc.tensor.matmul(
                        pt[:, :w],
                        lhsT=kh_sb[:, ep:ep + 1, :, j * P:(j + 1) * P]
                            .squeeze(1),
                        rhs=qt_sb[:, ep:ep + 1, :, q0 + off:q0 + off + w]
                            .squeeze(1),
                        start=(ep == 0),
                        stop=(ep == NEP - 1),
                        perf_mode=mybir.MatmulPerfMode.DoubleRow,
                    )
                if off == 0:
                    nc.vector.tensor_add(pt[:, :256], pt[:, :256], msk[:])
                nc.scalar.activation(
                    pr[:, off:off + w],
                    pt[:, :w],
                    mybir.ActivationFunctionType.Exp,
                    scale=float(1.0 / (np.sqrt(H) * 32.0)),
                    accum_out=accs[:, ci:ci + 1],
                )
            den = smpool.tile([P, 1], F32, tag=f"den{j}", name=f"den{j}")
            nc.vector.tensor_reduce(
                den[:], accs[:], axis=mybir.AxisListType.X, op=mybir.AluOpType.add
            )
            rec = smpool.tile([P, 1], F32, tag=f"rec{j}", name=f"rec{j}")
            nc.vector.reciprocal(rec[:], den[:])
            pr_sb.append(pr)
            recs.append(rec)

        # ---- phase B: vh[kl, h] (e-outer, 8 concurrent banks); each chunk is
        # scaled by the softmax reciprocal (from phase D) as it is copied ----
        vh_sb = [vhpool.tile([P, H], BF16, tag="vh", name=f"vh{j}")
                 for j in range(NKS)]
        for hb in range(H // NB):
            pts = [psum.tile([P, NB], F32, tag="ps", name=f"pp_vh{j}_{hb}")
                   for j in range(NKS)]
            for e in range(NE):
                for j in range(NKS):
                    nc.tensor.matmul(
                        pts[j][:],
                        lhsT=vt_sb[:, e * KL + j * P:e * KL + (j + 1) * P],
                        rhs=wv_sb[:, e * H + hb * NB:e * H + (hb + 1) * NB],
                        start=(e == 0),
                        stop=(e == NE - 1),
                    )
            for j in range(NKS):
                sl = vh_sb[j][:, hb * NB:(hb + 1) * NB]
                nc.vector.tensor_copy(sl, pts[j][:])
                nc.vector.tensor_scalar_mul(sl, sl, recs[j][:])

        # ---- phase E: PV + output ----
        # Interleave big-t (many accumulation MMs) and small-t (few) groups so
        # the DVE copy + out-DMA drain keeps pace with PSUM-bank production.
        # t=1,0 (fewest accumulation MMs) go last so the post-last-matmul
        # copy+DMA drain is as short as possible.
        pv_order = []
        lo, hi = 2, NQT - 1
        while lo <= hi:
            pv_order.append(hi)
            if lo < hi:
                pv_order.append(lo)
            hi -= 1
            lo += 1
        pv_order += [1, 0]
        for idx, t in enumerate(pv_order):
            jmax = t // 2
            for hb in range(H // NB):
                pt = psum.tile([P, NB], F32, tag="ps", name=f"pv{t}_{hb}")
                for j in range(jmax + 1):
                    off = t * P - 256 * j
                    nc.tensor.matmul(
                        pt[:],
                        lhsT=pr_sb[j][:, off:off + P],
                        rhs=vh_sb[j][:, hb * NB:(hb + 1) * NB],
                        start=(j == 0),
                        stop=(j == jmax),
                    )
                ot = ostpool.tile([P, NB], BF16, tag="ost", name=f"ot{t}_{hb}")
                if idx >= len(pv_order) - 2:
                    # final two tiles: parallelize the drain across engines
                    # (copies on DVE+ACT, descriptor gen on SP+gpsimd rings)
                    # so nothing serializes behind the last matmul.
                    if hb == 0:
                        nc.vector.tensor_copy(ot[:], pt[:])
                        deng = nc.sync
                    else:
                        nc.scalar.copy(ot[:], pt[:])
                        deng = nc.gpsimd
                    deng.dma_start(
                        out[t * P:(t + 1) * P, hb * NB:(hb + 1) * NB], ot[:]
                    )
                    continue
                nc.vector.tensor_copy(ot[:], pt[:])
                eng = nc.gpsimd if idx + 1 <= NQT - 4 else (
                    nc.sync if hb == 0 else nc.scalar)
                eng.dma_start(
                    out[t * P:(t + 1) * P, hb * NB:(hb + 1) * NB], ot[:]
                )

    nc.compile()
    return nc


# ---------------- host-side prep ----------------

def core_k_tiles(parity):
    return list(range(parity, 16, 2))


def _pack(m):
    """[NE*128, X] -> [128, NE*X]: partition-major so DMA descriptors are
    large and contiguous (row e*128+p, col x) -> (p, e*X+x)."""
    r, x = m.shape
    return np.ascontiguousarray(
        m.reshape(NE, P, x).transpose(1, 0, 2).reshape(P, NE * x)
    )


def _pack_ep(m):
    """[NE*128, X] -> [128, NE/2, 2, X] ep-major DoubleRow pairing:
    (p, ep, eo, x) <-> row (2*ep+eo)*128+p, col x."""
    r, x = m.shape
    return np.ascontiguousarray(
        m.reshape(NE // 2, 2, P, x).transpose(2, 0, 1, 3)
    )


def make_in_maps(q, k, v, Wq, Wk, Wv):
    """q,k,v: [4, 2048, 1024] f32; W*: [1024, 1024] f32 -> 8 per-core in_maps."""
    # merge the two score-side weights: scores = q (Wq^T Wk) k^T; M is
    # contracted into the k side on-device (khT = M k_c^T), so ship M^T
    # (contraction dim c on partitions).
    M = (Wq.T.astype(np.float64) @ Wk.astype(np.float64) * 32.0).astype(np.float32)
    wqT = _pack_ep(np.ascontiguousarray(M.T).astype(nfp8))
    wvT = _pack(Wv.T.astype(nbf16))

    kk = np.arange(P)[:, None]
    qq = np.arange(P)[None, :]
    tri = np.where(qq >= kk, 0.0, MASK_NEG).astype(np.float32)
    mask_even = np.concatenate([tri, np.zeros((P, P), np.float32)], axis=1)
    mask_odd = np.concatenate([np.full((P, P), MASK_NEG, np.float32), tri], axis=1)

    in_maps = []
    for c in range(8):
        b, parity = c // 2, c % 2
        rows = np.concatenate(
            [np.arange(g * P, (g + 1) * P) for g in core_k_tiles(parity)]
        )
        in_maps.append({
            "qT": _pack_ep(q[b].T.astype(nfp8)),
            "kT": _pack_ep(np.ascontiguousarray(k[b][rows].T).astype(nfp8)),
            "vT": _pack(v[b][rows].T.astype(nbf16)),
            "wqT": wqT,
            "wvT": wvT,
            "mask": mask_even if parity == 0 else mask_odd,
        })
    return in_maps


def combine_outputs(outs):
    """outs: list of 8 [2048, 1024] partial arrays -> [4, 2048, 1024]."""
    res = np.empty((4, SEQ, H), np.float32)
    for b in range(4):
        res[b] = outs[2 * b].astype(np.float32) + outs[2 * b + 1].astype(np.float32)
    return res


_NC_CACHE = []


def kernel(q, k, v, Wq, Wk, Wv):
    """Full inputs in, full output out; 8-core TRN2 SPMD inside."""
    q = np.asarray(q, dtype=np.float32)
    k = np.asarray(k, dtype=np.float32)
    v = np.asarray(v, dtype=np.float32)
    Wq = np.asarray(Wq, dtype=np.float32)
    Wk = np.asarray(Wk, dtype=np.float32)
    Wv = np.asarray(Wv, dtype=np.float32)

    if not _NC_CACHE:
        _NC_CACHE.append(build_nc())
    nc = _NC_CACHE[0]

    in_maps = make_in_maps(q, k, v, Wq, Wk, Wv)
    res = run_bass_kernel_spmd(nc, in_maps, core_ids=list(range(8)))
    outs = [res.results[i]["out"] for i in range(8)]
    return combine_outputs(outs)
